# revision 4
# baseline (speedup 1.0000x reference)
"""Capsule dynamic-routing kernel for 8 trn2 NeuronCores (Bass/Tile).

Sharding: input capsules (ci=2048) split 8 ways -> 256 ci per core.
Votes are generated on-chip (block-diagonal x @ W, fp16 in / fp32 psum),
stored SBUF-resident in fp16 with layout [part=(b,c8), free=(g,n,co)].
Each routing iteration does the two votes-sized einsums on DVE
(2x fp16 mode) + PE (ones-matmul contraction), with a 128KB AllReduce
of the preactivation across cores per iteration.

Self-contained: hardcodes all shapes; harness calls kernel(**inputs).
"""

import numpy as np
from contextlib import ExitStack

BS, CI, NI, CO, NO = 16, 2048, 16, 64, 32
NCORES = 8
CIL = CI // NCORES          # 256 input capsules per core
G = CIL // 8                # 32 groups of 8 ci
KF = NO * CO                # 2048 = votes free width per i, (n, co) order
ITERS = 3

_CACHE = {}


def _build_program():
    import concourse.bass as bass
    import concourse.bacc as bacc
    import concourse.tile as tile
    from concourse import mybir

    f16, f32 = mybir.dt.float16, mybir.dt.float32
    AF = mybir.ActivationFunctionType
    ALU = mybir.AluOpType
    X = mybir.AxisListType.X

    nc = bacc.Bacc(
        "TRN2", target_bir_lowering=False, debug=False, num_devices=NCORES
    )

    # host-prepared per-core inputs
    xbd_d = nc.dram_tensor("xbd", [128, G, 128], f16, kind="ExternalInput").ap()
    xT_d = nc.dram_tensor("xT", [128, G, 16], f16, kind="ExternalInput").ap()
    w_d = nc.dram_tensor("w", [G * 128, KF], f16, kind="ExternalInput").ap()
    bias_d = nc.dram_tensor("bias", [1, KF], f32, kind="ExternalInput").ap()
    out_d = nc.dram_tensor("out", [BS, KF], f32, kind="ExternalOutput").ap()

    # ones block-diagonal (delta_{b,b'} at partition b*8+c8) for the
    # i-contraction matmul, embedded in the NEFF
    ones_np = np.zeros((128, BS), dtype=np.float16)
    for b in range(BS):
        ones_np[b * 8 : (b + 1) * 8, b] = 1.0
    ones_d = nc.inline_tensor(ones_np, name="onesbd").ap()

    groups = [list(range(NCORES))]
    ar_in = [nc.dram_tensor(f"ar_in{i}", [BS, KF], f32).ap() for i in range(ITERS)]
    ar_out = [
        nc.dram_tensor(f"ar_out{i}", [BS, KF], f32, addr_space="Shared").ap()
        for i in range(ITERS)
    ]

    with tile.TileContext(nc) as tc, ExitStack() as ctx:
        # ---- persistent pools ----
        pers = ctx.enter_context(tc.tile_pool(name="pers", bufs=1))
        votes = pers.tile([128, G * KF], f16, tag="votes")     # 128KB/part
        logits = pers.tile([128, G * CO], f32, tag="logits")   # 8KB/part
        bias_rep = pers.tile([128, KF], f32, tag="bias")       # 8KB
        ones_sb = pers.tile([128, BS], f16, tag="ones")
        act32 = pers.tile([128, KF], f32, tag="act32")         # 8KB
        act16 = pers.tile([128, KF], f16, tag="act16")         # 4KB
        scr32 = pers.tile([128, KF], f32, tag="scr32")         # 8KB scratch
        nrm = pers.tile([128, CO], f32, tag="nrm")
        den = pers.tile([128, CO], f32, tag="den")
        sden = pers.tile([128, G], f32, tag="sden")
        route16 = pers.tile([128, G * CO], f16, tag="route")   # 4KB
        arst = pers.tile([BS, KF], f32, tag="arst")

        nc.sync.dma_start(ones_sb[:], ones_d)
        nc.sync.dma_start(
            bias_rep[:],
            bias_d.broadcast_to((128, KF)),
        )
        nc.vector.memset(logits[:], 0.0)

        ppool = ctx.enter_context(tc.tile_pool(name="psum", bufs=1, space="PSUM"))
        vpsum = ctx.enter_context(tc.tile_pool(name="vpsum", bufs=2, space="PSUM"))

        # ================= pass 0: votes + uniform-route preactivation ====
        pre_ps = ppool.tile([BS, KF], f32, tag="pre")
        with tc.tile_pool(name="p0", bufs=1) as p0pool, tc.tile_pool(
            name="wstream", bufs=3
        ) as wpool:
            xbd_sb = p0pool.tile([128, G * 128], f16, tag="xbd")
            xT_sb = p0pool.tile([128, G * 16], f16, tag="xT")
            nc.sync.dma_start(
                xbd_sb[:].rearrange("p (g m) -> p g m", g=G), xbd_d
            )
            nc.sync.dma_start(xT_sb[:].rearrange("p (g m) -> p g m", g=G), xT_d)

            for g in range(G):
                w_t = wpool.tile([128, KF], f16, tag="w")
                nc.sync.dma_start(w_t[:], w_d[g * 128 : (g + 1) * 128, :])
                lhs_v = xbd_sb[:, g * 128 : (g + 1) * 128]
                lhs_p = xT_sb[:, g * 16 : (g + 1) * 16]
                for h in range(2):
                    pv = vpsum.tile([128, 1024], f32, tag="pv")
                    for q in range(2):
                        col = h * 1024 + q * 512
                        nc.tensor.matmul(
                            pv[:, q * 512 : (q + 1) * 512],
                            lhs_v,
                            w_t[:, col : col + 512],
                            start=True,
                            stop=True,
                        )
                    # evacuate fp32 psum -> fp16 votes slice (alternate engines)
                    dst = votes[:, g * KF + h * 1024 : g * KF + (h + 1) * 1024]
                    if h == 0:
                        nc.vector.tensor_copy(dst, pv[:])
                    else:
                        nc.scalar.copy(dst, pv[:])
                for q in range(4):
                    nc.tensor.matmul(
                        pre_ps[:, q * 512 : (q + 1) * 512],
                        lhs_p,
                        w_t[:, q * 512 : (q + 1) * 512],
                        start=(g == 0),
                        stop=(g == G - 1),
                        skip_group_check=True,
                    )

        # ============ allreduce + squash (shared by all iterations) =======
        def allreduce_squash(it, scale):
            # pre_ps [BS, KF] fp32 -> arst -> DRAM -> AllReduce -> replicate
            nc.scalar.activation(arst[:], pre_ps[:], AF.Copy, scale=scale)
            nc.sync.dma_start(ar_in[it], arst[:])
            nc.gpsimd.collective_compute(
                "AllReduce",
                ALU.add,
                replica_groups=groups,
                ins=[ar_in[it]],
                outs=[ar_out[it]],
            )
            for b in range(BS):
                nc.sync.dma_start(
                    act32[b * 8 : (b + 1) * 8, :],
                    ar_out[it][b : b + 1, :].broadcast_to((8, KF)),
                )
            # + bias, then squash along n (n is the outer free dim: tree)
            nc.vector.tensor_add(act32[:], act32[:], bias_rep[:])
            nc.vector.tensor_mul(scr32[:], act32[:], act32[:])
            s4 = scr32[:].rearrange("p (n c) -> p n c", n=NO)
            for lv in (16, 8, 4, 2, 1):
                nc.vector.tensor_add(
                    s4[:, 0:lv, :], s4[:, 0:lv, :], s4[:, lv : 2 * lv, :]
                )
            # nrm = sum pre^2 ; factor = sqrt(s)/(1+s)
            nc.scalar.sqrt(nrm[:], s4[:, 0, :])
            nc.vector.tensor_scalar_add(den[:], s4[:, 0, :], 1.0)
            nc.vector.reciprocal(den[:], den[:])
            nc.vector.tensor_mul(nrm[:], nrm[:], den[:])
            a4 = act32[:].rearrange("p (n c) -> p n c", n=NO)
            nc.vector.tensor_mul(
                a4,
                a4,
                nrm[:].rearrange("p (u c) -> p u c", u=1).broadcast_to((128, NO, CO)),
            )
            nc.scalar.copy(act16[:], act32[:])

        allreduce_squash(0, 1.0 / CO)

        # ===================== routing iterations =========================
        with tc.tile_pool(name="wv", bufs=2) as wvpool:
            for it in range(1, ITERS):
                # ---- distances: logits += sum_n votes * act ----
                for mc in range(G // 2):  # macro-chunks of 2 groups
                    wv = wvpool.tile([128, 2 * KF], f16, tag="wv")
                    wv4 = wv[:].rearrange("p (g n c) -> p g n c", g=2, n=NO)
                    v4 = votes[
                        :, mc * 2 * KF : (mc + 1) * 2 * KF
                    ].rearrange("p (g n c) -> p g n c", g=2, n=NO)
                    a_b = (
                        act16[:]
                        .rearrange("p (u n c) -> p u n c", u=1, n=NO)
                        .broadcast_to((128, 2, NO, CO))
                    )
                    nc.vector.tensor_mul(wv4, v4, a_b)
                    for lv in (16, 8, 4, 2, 1):
                        nc.vector.tensor_add(
                            wv4[:, :, 0:lv, :],
                            wv4[:, :, 0:lv, :],
                            wv4[:, :, lv : 2 * lv, :],
                        )
                    lg = logits[
                        :, mc * 2 * CO : (mc + 1) * 2 * CO
                    ].rearrange("p (g c) -> p g c", g=2)
                    nc.vector.tensor_add(lg, lg, wv4[:, :, 0, :])
                # ---- softmax over co (free innermost) ----
                nc.scalar.activation(scr32[:, : G * CO], logits[:], AF.Exp)
                nc.vector.tensor_reduce(
                    sden[:],
                    scr32[:, : G * CO].rearrange("p (g c) -> p g c", g=G),
                    axis=X,
                    op=ALU.add,
                )
                nc.vector.reciprocal(sden[:], sden[:])
                nc.vector.tensor_mul(
                    route16[:].rearrange("p (g c) -> p g c", g=G),
                    scr32[:, : G * CO].rearrange("p (g c) -> p g c", g=G),
                    sden[:].rearrange("p (g u) -> p g u", u=1).broadcast_to((128, G, CO)),
                )
                # ---- preactivation: pre = sum_i route * votes ----
                pre_ps2 = ppool.tile([BS, KF], f32, tag="pre")
                for g in range(G):
                    rv = wvpool.tile([128, KF], f16, tag="wv")
                    nc.vector.tensor_mul(
                        rv[:].rearrange("p (n c) -> p n c", n=NO),
                        votes[:, g * KF : (g + 1) * KF].rearrange(
                            "p (n c) -> p n c", n=NO
                        ),
                        route16[:, g * CO : (g + 1) * CO]
                        .rearrange("p (u c) -> p u c", u=1)
                        .broadcast_to((128, NO, CO)),
                    )
                    for q in range(4):
                        nc.tensor.matmul(
                            pre_ps2[:, q * 512 : (q + 1) * 512],
                            ones_sb[:],
                            rv[:, q * 512 : (q + 1) * 512],
                            start=(g == 0),
                            stop=(g == G - 1),
                            skip_group_check=True,
                        )
                pre_ps = pre_ps2
                allreduce_squash(it, 1.0)

        # final activation -> DRAM (partitions b*8, i.e. c8==0)
        nc.sync.dma_start(
            out_d, act32[:].rearrange("(b c) k -> b c k", c=8)[:, 0, :]
        )

    nc.compile()
    return nc


def _prep_inputs(x, weight, bias):
    """Per-core host-side shard prep. Returns list of in_maps."""
    w_nc = (
        weight.reshape(CI, NI, CO, NO)
        .transpose(0, 1, 3, 2)
        .reshape(CI * NI, NO * CO)
        .astype(np.float16)
    )
    bias_nc = np.ascontiguousarray(
        bias.reshape(CO, NO).T.reshape(NO * CO)
    ).astype(np.float32)
    in_maps = []
    for c in range(NCORES):
        xs = x[:, c * CIL : (c + 1) * CIL, :].astype(np.float16)  # [BS, CIL, NI]
        # xbd[p=(c8,ni), g, m=(b,c8')] block-diagonal
        xbd = np.zeros((128, G, 128), dtype=np.float16)
        xg = xs.reshape(BS, G, 8, NI)  # b g c8 ni
        for c8 in range(8):
            # rows c8*16..c8*16+16, cols b*8+c8
            xbd[c8 * NI : (c8 + 1) * NI, :, c8::8] = xg[:, :, c8, :].transpose(
                2, 1, 0
            )
        # xT[p=(c8,ni), g, b] = x[b, g*8+c8, ni]
        xT = np.ascontiguousarray(
            xg.transpose(2, 3, 1, 0).reshape(128, G, BS)
        ).astype(np.float16)
        ws = np.ascontiguousarray(
            w_nc[c * CIL * NI : (c + 1) * CIL * NI, :]
        )
        in_maps.append(
            {"xbd": xbd, "xT": xT, "w": ws, "bias": bias_nc}
        )
    return in_maps


def _run(inputs, trace=False, **kw):
    from concourse.bass_utils import run_bass_kernel_spmd

    if "nc" not in _CACHE:
        _CACHE["nc"] = _build_program()
    nc = _CACHE["nc"]
    in_maps = _prep_inputs(inputs["x"], inputs["weight"], inputs["bias"])
    res = run_bass_kernel_spmd(
        nc, in_maps, core_ids=list(range(NCORES)), trace=trace, **kw
    )
    out = res.results[0]["out"]  # [BS, NO*CO] in (n, co) order
    out = np.ascontiguousarray(
        out.reshape(BS, NO, CO).transpose(0, 2, 1)
    ).astype(np.float32)
    return out, res


def kernel(**inputs):
    out, _ = _run(inputs, trace=False)
    return out


# revision 12
# speedup vs baseline: 1.2002x; 1.2002x over previous
"""Capsule dynamic-routing kernel for 8 trn2 NeuronCores (Bass/Tile).

Sharding: input capsules (ci=2048) split 8 ways -> 256 ci per core.
Votes are generated on-chip (block-diagonal x @ W, fp16 in / fp32 psum),
stored SBUF-resident in fp16 with layout [part=(b,c8), free=(g,n,co)].
Each routing iteration does the two votes-sized einsums on DVE
(2x fp16 mode) + PE (ones-matmul contraction), with a 128KB AllReduce
of the preactivation across cores per iteration.

Self-contained: hardcodes all shapes; harness calls kernel(**inputs).
"""

import numpy as np
from contextlib import ExitStack

BS, CI, NI, CO, NO = 16, 2048, 16, 64, 32
NCORES = 8
CIL = CI // NCORES          # 256 input capsules per core
G = CIL // 8                # 32 groups of 8 ci
KF = NO * CO                # 2048 = votes free width per i, (n, co) order
ITERS = 3

_CACHE = {}


def _build_program():
    import concourse.bass as bass
    import concourse.bacc as bacc
    import concourse.tile as tile
    from concourse import mybir

    f16, f32 = mybir.dt.float16, mybir.dt.float32
    AF = mybir.ActivationFunctionType
    ALU = mybir.AluOpType
    X = mybir.AxisListType.X

    nc = bacc.Bacc(
        "TRN2", target_bir_lowering=False, debug=False, num_devices=NCORES
    )

    # host-prepared per-core inputs
    xbd_d = nc.dram_tensor("xbd", [128, G, 128], f16, kind="ExternalInput").ap()
    xT_d = nc.dram_tensor("xT", [128, G, 16], f16, kind="ExternalInput").ap()
    w_d = nc.dram_tensor("w", [G * 128, KF], f16, kind="ExternalInput").ap()
    bias_d = nc.dram_tensor("bias", [1, KF], f32, kind="ExternalInput").ap()
    out_d = nc.dram_tensor("out", [BS, KF], f32, kind="ExternalOutput").ap()

    # ones block-diagonal (delta_{b,b'} at partition b*8+c8) for the
    # i-contraction matmul, embedded in the NEFF
    ones_np = np.zeros((128, BS), dtype=np.float16)
    for b in range(BS):
        ones_np[b * 8 : (b + 1) * 8, b] = 1.0
    ones_d = nc.inline_tensor(ones_np, name="onesbd").ap()

    groups = [list(range(NCORES))]
    ar_in = [nc.dram_tensor(f"ar_in{i}", [BS, KF], f32).ap() for i in range(ITERS)]
    ar_out = [
        nc.dram_tensor(f"ar_out{i}", [BS, KF], f32, addr_space="Shared").ap()
        for i in range(ITERS)
    ]
    # tiny warmup collective: pays the CC-core one-time init during pass 0
    warm_in = nc.inline_tensor(np.zeros((1, 16), np.float32), name="warm_in").ap()
    warm_out = nc.dram_tensor("warm_out", [1, 16], f32, addr_space="Shared").ap()

    with tile.TileContext(nc) as tc, ExitStack() as ctx:
        # ---- persistent pools ----
        pers = ctx.enter_context(tc.tile_pool(name="pers", bufs=1))
        votes = pers.tile([128, G * KF], f16, tag="votes")     # 128KB/part
        logits = pers.tile([128, G * CO], f32, tag="logits")   # 8KB/part
        bias_rep = pers.tile([128, KF], f32, tag="bias")       # 8KB
        ones_sb = pers.tile([128, BS], f16, tag="ones")
        act32 = pers.tile([128, KF], f32, tag="act32")         # 8KB
        act16 = pers.tile([128, KF], f16, tag="act16")         # 4KB
        scr32 = pers.tile([128, KF], f32, tag="scr32")         # 8KB scratch
        nrm = pers.tile([128, CO], f32, tag="nrm")
        den = pers.tile([128, CO], f32, tag="den")
        sden = pers.tile([128, G], f32, tag="sden")
        route16 = pers.tile([128, G * CO], f16, tag="route")   # 4KB

        nc.gpsimd.collective_compute(
            "AllReduce",
            ALU.add,
            replica_groups=groups,
            ins=[warm_in],
            outs=[warm_out],
        )
        nc.sync.dma_start(ones_sb[:], ones_d)
        nc.sync.dma_start(
            bias_rep[:],
            bias_d.broadcast_to((128, KF)),
        )
        nc.vector.memset(logits[:], 0.0)

        ppool = ctx.enter_context(tc.tile_pool(name="psum", bufs=1, space="PSUM"))
        vpsum = ctx.enter_context(tc.tile_pool(name="vpsum", bufs=4, space="PSUM"))

        # ================= pass 0: votes + uniform-route preactivation ====
        pre_ps = ppool.tile([BS, KF], f32, tag="pre")
        with tc.tile_pool(name="p0", bufs=1) as p0pool, tc.tile_pool(
            name="wstream", bufs=3
        ) as wpool:
            xbd_sb = p0pool.tile([128, G * 128], f16, tag="xbd")
            xT_sb = p0pool.tile([128, G * 16], f16, tag="xT")
            nc.sync.dma_start(
                xbd_sb[:].rearrange("p (g m) -> p g m", g=G), xbd_d
            )
            nc.sync.dma_start(xT_sb[:].rearrange("p (g m) -> p g m", g=G), xT_d)

            for g in range(G):
                w_t = wpool.tile([128, KF], f16, tag="w")
                nc.sync.dma_start(w_t[:], w_d[g * 128 : (g + 1) * 128, :])
                lhs_v = xbd_sb[:, g * 128 : (g + 1) * 128]
                lhs_p = xT_sb[:, g * 16 : (g + 1) * 16]
                for h in range(4):
                    pv = vpsum.tile([128, 512], f32, tag="pv")
                    col = h * 512
                    nc.tensor.matmul(
                        pv[:],
                        lhs_v,
                        w_t[:, col : col + 512],
                        start=True,
                        stop=True,
                    )
                    # evacuate fp32 psum -> fp16 votes slice (alternate engines)
                    dst = votes[:, g * KF + col : g * KF + col + 512]
                    if h % 2 == 0:
                        nc.vector.tensor_copy(dst, pv[:])
                    else:
                        nc.scalar.copy(dst, pv[:])
                for q in range(4):
                    nc.tensor.matmul(
                        pre_ps[:, q * 512 : (q + 1) * 512],
                        lhs_p,
                        w_t[:, q * 512 : (q + 1) * 512],
                        start=(g == 0),
                        stop=(g == G - 1),
                        skip_group_check=True,
                    )

        # ============ allreduce + squash (shared by all iterations) =======
        def allreduce_squash(it, scale):
            # pre_ps [BS, KF] fp32 -> scr32 rows 0:16 -> DRAM -> AllReduce
            nc.scalar.activation(scr32[0:BS, :], pre_ps[:], AF.Copy, scale=scale)
            nc.sync.dma_start(ar_in[it], scr32[0:BS, :])
            nc.gpsimd.collective_compute(
                "AllReduce",
                ALU.add,
                replica_groups=groups,
                ins=[ar_in[it]],
                outs=[ar_out[it]],
            )
            for b in range(BS):
                nc.sync.dma_start(
                    act32[b * 8 : (b + 1) * 8, :],
                    ar_out[it][b : b + 1, :].broadcast_to((8, KF)),
                )
            # + bias, then squash along n (n is the outer free dim: tree)
            nc.vector.tensor_add(act32[:], act32[:], bias_rep[:])
            nc.scalar.activation(scr32[:], act32[:], AF.Square)
            s4 = scr32[:].rearrange("p (n c) -> p n c", n=NO)
            for lv in (16, 8, 4, 2, 1):
                nc.vector.tensor_add(
                    s4[:, 0:lv, :], s4[:, 0:lv, :], s4[:, lv : 2 * lv, :]
                )
            # nrm = sum pre^2 ; factor = sqrt(s)/(1+s)
            nc.scalar.sqrt(nrm[:], s4[:, 0, :])
            nc.vector.tensor_scalar_add(den[:], s4[:, 0, :], 1.0)
            nc.vector.reciprocal(den[:], den[:])
            nc.vector.tensor_mul(nrm[:], nrm[:], den[:])
            a4 = act32[:].rearrange("p (n c) -> p n c", n=NO)
            nc.vector.tensor_mul(
                a4,
                a4,
                nrm[:].rearrange("p (u c) -> p u c", u=1).broadcast_to((128, NO, CO)),
            )
            nc.scalar.copy(act16[:], act32[:])

        allreduce_squash(0, 1.0 / CO)

        # ===================== routing iterations =========================
        MCG = 4  # groups per dist macro-chunk
        with tc.tile_pool(name="wv", bufs=2) as wvpool:
            for it in range(1, ITERS):
                # ---- distances: logits += sum_n votes * act ----
                for mc in range(G // MCG):
                    wv = wvpool.tile([128, MCG * KF], f16, tag="wv")
                    wv4 = wv[:].rearrange("p (g n c) -> p g n c", g=MCG, n=NO)
                    v4 = votes[
                        :, mc * MCG * KF : (mc + 1) * MCG * KF
                    ].rearrange("p (g n c) -> p g n c", g=MCG, n=NO)
                    a_b = (
                        act16[:]
                        .rearrange("p (u n c) -> p u n c", u=1, n=NO)
                        .broadcast_to((128, MCG, NO, CO))
                    )
                    nc.vector.tensor_mul(wv4, v4, a_b)
                    for lv in (16, 8, 4, 2, 1):
                        nc.vector.tensor_add(
                            wv4[:, :, 0:lv, :],
                            wv4[:, :, 0:lv, :],
                            wv4[:, :, lv : 2 * lv, :],
                        )
                    lg = logits[
                        :, mc * MCG * CO : (mc + 1) * MCG * CO
                    ].rearrange("p (g c) -> p g c", g=MCG)
                    nc.vector.tensor_add(lg, lg, wv4[:, :, 0, :])
                # ---- softmax over co (free innermost) ----
                nc.scalar.activation(scr32[:, : G * CO], logits[:], AF.Exp)
                nc.vector.tensor_reduce(
                    sden[:],
                    scr32[:, : G * CO].rearrange("p (g c) -> p g c", g=G),
                    axis=X,
                    op=ALU.add,
                )
                nc.vector.reciprocal(sden[:], sden[:])
                nc.vector.tensor_mul(
                    route16[:].rearrange("p (g c) -> p g c", g=G),
                    scr32[:, : G * CO].rearrange("p (g c) -> p g c", g=G),
                    sden[:].rearrange("p (g u) -> p g u", u=1).broadcast_to((128, G, CO)),
                )
                # ---- preactivation: pre = sum_i route * votes ----
                pre_ps2 = ppool.tile([BS, KF], f32, tag="pre")
                for g2 in range(G // 2):
                    rv = wvpool.tile([128, 2 * KF], f16, tag="wv")
                    nc.vector.tensor_mul(
                        rv[:].rearrange("p (g n c) -> p g n c", g=2, n=NO),
                        votes[:, g2 * 2 * KF : (g2 + 1) * 2 * KF].rearrange(
                            "p (g n c) -> p g n c", g=2, n=NO
                        ),
                        route16[:, g2 * 2 * CO : (g2 + 1) * 2 * CO]
                        .rearrange("p (g u c) -> p g u c", g=2, u=1)
                        .broadcast_to((128, 2, NO, CO)),
                    )
                    for q in range(8):
                        nc.tensor.matmul(
                            pre_ps2[:, (q % 4) * 512 : (q % 4 + 1) * 512],
                            ones_sb[:],
                            rv[:, q * 512 : (q + 1) * 512],
                            start=(g2 == 0 and q < 4),
                            stop=(g2 == G // 2 - 1 and q >= 4),
                            skip_group_check=True,
                        )
                pre_ps = pre_ps2
                allreduce_squash(it, 1.0)

        # final activation -> DRAM (partitions b*8, i.e. c8==0)
        nc.sync.dma_start(
            out_d, act32[:].rearrange("(b c) k -> b c k", c=8)[:, 0, :]
        )

    nc.compile()
    return nc


def _prep_inputs(x, weight, bias):
    """Per-core host-side shard prep. Returns list of in_maps."""
    w_nc = (
        weight.reshape(CI, NI, CO, NO)
        .transpose(0, 1, 3, 2)
        .reshape(CI * NI, NO * CO)
        .astype(np.float16)
    )
    bias_nc = np.ascontiguousarray(
        bias.reshape(CO, NO).T.reshape(NO * CO)
    ).astype(np.float32)
    in_maps = []
    for c in range(NCORES):
        xs = x[:, c * CIL : (c + 1) * CIL, :].astype(np.float16)  # [BS, CIL, NI]
        # xbd[p=(c8,ni), g, m=(b,c8')] block-diagonal
        xbd = np.zeros((128, G, 128), dtype=np.float16)
        xg = xs.reshape(BS, G, 8, NI)  # b g c8 ni
        for c8 in range(8):
            # rows c8*16..c8*16+16, cols b*8+c8
            xbd[c8 * NI : (c8 + 1) * NI, :, c8::8] = xg[:, :, c8, :].transpose(
                2, 1, 0
            )
        # xT[p=(c8,ni), g, b] = x[b, g*8+c8, ni]
        xT = np.ascontiguousarray(
            xg.transpose(2, 3, 1, 0).reshape(128, G, BS)
        ).astype(np.float16)
        ws = np.ascontiguousarray(
            w_nc[c * CIL * NI : (c + 1) * CIL * NI, :]
        )
        in_maps.append(
            {"xbd": xbd, "xT": xT, "w": ws, "bias": bias_nc}
        )
    return in_maps


def _run(inputs, trace=False, **kw):
    from concourse.bass_utils import run_bass_kernel_spmd

    if "nc" not in _CACHE:
        _CACHE["nc"] = _build_program()
    nc = _CACHE["nc"]
    in_maps = _prep_inputs(inputs["x"], inputs["weight"], inputs["bias"])
    res = run_bass_kernel_spmd(
        nc, in_maps, core_ids=list(range(NCORES)), trace=trace, **kw
    )
    out = res.results[0]["out"]  # [BS, NO*CO] in (n, co) order
    out = np.ascontiguousarray(
        out.reshape(BS, NO, CO).transpose(0, 2, 1)
    ).astype(np.float32)
    return out, res


def kernel(**inputs):
    out, _ = _run(inputs, trace=False)
    return out


# revision 14
# speedup vs baseline: 1.3374x; 1.1143x over previous
"""Capsule dynamic-routing kernel for 8 trn2 NeuronCores (Bass/Tile).

Sharding: input capsules (ci=2048) split 8 ways -> 256 ci per core.
Votes are generated on-chip (block-diagonal x @ W, fp16 in / fp32 psum),
stored SBUF-resident in fp16 with layout [part=(b,c8), free=(g,n,co)].
Each routing iteration does the two votes-sized einsums on DVE
(2x fp16 mode) + PE (ones-matmul contraction), with a 128KB AllReduce
of the preactivation across cores per iteration.

Self-contained: hardcodes all shapes; harness calls kernel(**inputs).
"""

import numpy as np
from contextlib import ExitStack

BS, CI, NI, CO, NO = 16, 2048, 16, 64, 32
NCORES = 8
CIL = CI // NCORES          # 256 input capsules per core
G = CIL // 8                # 32 groups of 8 ci
KF = NO * CO                # 2048 = votes free width per i, (n, co) order
ITERS = 3

_CACHE = {}


def _build_program():
    import concourse.bass as bass
    import concourse.bacc as bacc
    import concourse.tile as tile
    from concourse import mybir

    f16, f32 = mybir.dt.float16, mybir.dt.float32
    AF = mybir.ActivationFunctionType
    ALU = mybir.AluOpType
    X = mybir.AxisListType.X

    nc = bacc.Bacc(
        "TRN2", target_bir_lowering=False, debug=False, num_devices=NCORES
    )

    # host-prepared per-core inputs
    xbd_d = nc.dram_tensor("xbd", [128, G, 128], f16, kind="ExternalInput").ap()
    xT_d = nc.dram_tensor("xT", [128, G, 16], f16, kind="ExternalInput").ap()
    w_d = nc.dram_tensor("w", [G * 128, KF], f16, kind="ExternalInput").ap()
    bias_d = nc.dram_tensor("bias", [1, KF], f32, kind="ExternalInput").ap()
    out_d = nc.dram_tensor("out", [BS, KF], f32, kind="ExternalOutput").ap()

    # ones block-diagonal (delta_{b,b'} at partition b*8+c8) for the
    # i-contraction matmul, embedded in the NEFF
    ones_np = np.zeros((128, BS), dtype=np.float16)
    for b in range(BS):
        ones_np[b * 8 : (b + 1) * 8, b] = 1.0
    ones_d = nc.inline_tensor(ones_np, name="onesbd").ap()

    groups = [list(range(NCORES))]
    ar_in = [nc.dram_tensor(f"ar_in{i}", [BS, KF], f16).ap() for i in range(ITERS)]
    ar_out = [
        nc.dram_tensor(f"ar_out{i}", [BS, KF], f16, addr_space="Shared").ap()
        for i in range(ITERS)
    ]
    # tiny warmup collective: pays the CC-core one-time init during pass 0
    warm_in = nc.inline_tensor(np.zeros((1, 16), np.float32), name="warm_in").ap()
    warm_out = nc.dram_tensor("warm_out", [1, 16], f32, addr_space="Shared").ap()

    with tile.TileContext(nc) as tc, ExitStack() as ctx:
        # ---- persistent pools ----
        pers = ctx.enter_context(tc.tile_pool(name="pers", bufs=1))
        votes = pers.tile([128, G * KF], f16, tag="votes")     # 128KB/part
        logits = pers.tile([128, G * CO], f32, tag="logits")   # 8KB/part
        bias_rep = pers.tile([128, KF], f32, tag="bias")       # 8KB
        ones_sb = pers.tile([128, BS], f16, tag="ones")
        act32 = pers.tile([128, KF], f32, tag="act32")         # 8KB
        act16 = pers.tile([128, KF], f16, tag="act16")         # 4KB
        scr32 = pers.tile([128, KF], f32, tag="scr32")         # 8KB scratch
        nrm = pers.tile([128, CO], f32, tag="nrm")
        den = pers.tile([128, CO], f32, tag="den")
        sden = pers.tile([128, G], f32, tag="sden")
        route16 = pers.tile([128, G * CO], f16, tag="route")   # 4KB
        arst16 = pers.tile([BS, KF], f16, tag="arst16")        # 4KB

        nc.gpsimd.collective_compute(
            "AllReduce",
            ALU.add,
            replica_groups=groups,
            ins=[warm_in],
            outs=[warm_out],
        )
        nc.sync.dma_start(ones_sb[:], ones_d)
        nc.sync.dma_start(
            bias_rep[:],
            bias_d.broadcast_to((128, KF)),
        )
        nc.vector.memset(logits[:], 0.0)

        ppool = ctx.enter_context(tc.tile_pool(name="psum", bufs=1, space="PSUM"))
        vpsum = ctx.enter_context(tc.tile_pool(name="vpsum", bufs=4, space="PSUM"))

        # ================= pass 0: votes + uniform-route preactivation ====
        pre_ps = ppool.tile([BS, KF], f32, tag="pre")
        with tc.tile_pool(name="p0", bufs=1) as p0pool, tc.tile_pool(
            name="wstream", bufs=5
        ) as wpool:
            xbd_sb = p0pool.tile([128, G * 128], f16, tag="xbd")
            xT_sb = p0pool.tile([128, G * 16], f16, tag="xT")
            nc.sync.dma_start(
                xbd_sb[:].rearrange("p (g m) -> p g m", g=G), xbd_d
            )
            nc.sync.dma_start(xT_sb[:].rearrange("p (g m) -> p g m", g=G), xT_d)

            for g in range(G):
                w_t = wpool.tile([128, KF], f16, tag="w")
                nc.sync.dma_start(w_t[:], w_d[g * 128 : (g + 1) * 128, :])
                lhs_v = xbd_sb[:, g * 128 : (g + 1) * 128]
                lhs_p = xT_sb[:, g * 16 : (g + 1) * 16]
                for h in range(4):
                    pv = vpsum.tile([128, 512], f32, tag="pv")
                    col = h * 512
                    nc.tensor.matmul(
                        pv[:],
                        lhs_v,
                        w_t[:, col : col + 512],
                        start=True,
                        stop=True,
                    )
                    # evacuate fp32 psum -> fp16 votes slice (alternate engines)
                    dst = votes[:, g * KF + col : g * KF + col + 512]
                    if h % 2 == 0:
                        nc.vector.tensor_copy(dst, pv[:])
                    else:
                        nc.scalar.copy(dst, pv[:])
                for q in range(4):
                    nc.tensor.matmul(
                        pre_ps[:, q * 512 : (q + 1) * 512],
                        lhs_p,
                        w_t[:, q * 512 : (q + 1) * 512],
                        start=(g == 0),
                        stop=(g == G - 1),
                        skip_group_check=True,
                    )

        # ============ allreduce + squash (shared by all iterations) =======
        def allreduce_squash(it, scale):
            # pre_ps [BS, KF] fp32 -> f16 staging -> DRAM -> AllReduce (f16)
            nc.scalar.activation(arst16[:], pre_ps[:], AF.Copy, scale=scale)
            nc.sync.dma_start(ar_in[it], arst16[:])
            nc.gpsimd.collective_compute(
                "AllReduce",
                ALU.add,
                replica_groups=groups,
                ins=[ar_in[it]],
                outs=[ar_out[it]],
            )
            # replicate 8x across partitions in one DMA (into act16 tile)
            nc.sync.dma_start(
                act16[:],
                ar_out[it]
                .rearrange("b (u k) -> b u k", u=1)
                .broadcast_to((BS, 8, KF)),
            )
            # + bias, then squash along n (n is the outer free dim: tree)
            nc.vector.tensor_add(act32[:], act16[:], bias_rep[:])
            nc.scalar.activation(scr32[:], act32[:], AF.Square)
            s4 = scr32[:].rearrange("p (n c) -> p n c", n=NO)
            for lv in (16, 8, 4, 2, 1):
                nc.vector.tensor_add(
                    s4[:, 0:lv, :], s4[:, 0:lv, :], s4[:, lv : 2 * lv, :]
                )
            # nrm = sum pre^2 ; factor = sqrt(s)/(1+s)
            nc.scalar.sqrt(nrm[:], s4[:, 0, :])
            nc.vector.tensor_scalar_add(den[:], s4[:, 0, :], 1.0)
            nc.vector.reciprocal(den[:], den[:])
            nc.vector.tensor_mul(nrm[:], nrm[:], den[:])
            a4 = act32[:].rearrange("p (n c) -> p n c", n=NO)
            nc.vector.tensor_mul(
                a4,
                a4,
                nrm[:].rearrange("p (u c) -> p u c", u=1).broadcast_to((128, NO, CO)),
            )
            nc.scalar.copy(act16[:], act32[:])

        allreduce_squash(0, 1.0 / CO)

        # ===================== routing iterations =========================
        MCG = 4  # groups per dist macro-chunk
        with tc.tile_pool(name="wv", bufs=2) as wvpool:
            for it in range(1, ITERS):
                # ---- distances: logits += sum_n votes * act ----
                for mc in range(G // MCG):
                    wv = wvpool.tile([128, MCG * KF], f16, tag="wv")
                    wv4 = wv[:].rearrange("p (g n c) -> p g n c", g=MCG, n=NO)
                    v4 = votes[
                        :, mc * MCG * KF : (mc + 1) * MCG * KF
                    ].rearrange("p (g n c) -> p g n c", g=MCG, n=NO)
                    a_b = (
                        act16[:]
                        .rearrange("p (u n c) -> p u n c", u=1, n=NO)
                        .broadcast_to((128, MCG, NO, CO))
                    )
                    nc.vector.tensor_mul(wv4, v4, a_b)
                    for lv in (16, 8, 4, 2, 1):
                        nc.vector.tensor_add(
                            wv4[:, :, 0:lv, :],
                            wv4[:, :, 0:lv, :],
                            wv4[:, :, lv : 2 * lv, :],
                        )
                    lg = logits[
                        :, mc * MCG * CO : (mc + 1) * MCG * CO
                    ].rearrange("p (g c) -> p g c", g=MCG)
                    nc.vector.tensor_add(lg, lg, wv4[:, :, 0, :])
                # ---- softmax over co (free innermost) ----
                nc.scalar.activation(scr32[:, : G * CO], logits[:], AF.Exp)
                nc.vector.tensor_reduce(
                    sden[:],
                    scr32[:, : G * CO].rearrange("p (g c) -> p g c", g=G),
                    axis=X,
                    op=ALU.add,
                )
                nc.vector.reciprocal(sden[:], sden[:])
                nc.vector.tensor_mul(
                    route16[:].rearrange("p (g c) -> p g c", g=G),
                    scr32[:, : G * CO].rearrange("p (g c) -> p g c", g=G),
                    sden[:].rearrange("p (g u) -> p g u", u=1).broadcast_to((128, G, CO)),
                )
                # ---- preactivation: pre = sum_i route * votes ----
                pre_ps2 = ppool.tile([BS, KF], f32, tag="pre")
                for g2 in range(G // 2):
                    rv = wvpool.tile([128, 2 * KF], f16, tag="wv")
                    nc.vector.tensor_mul(
                        rv[:].rearrange("p (g n c) -> p g n c", g=2, n=NO),
                        votes[:, g2 * 2 * KF : (g2 + 1) * 2 * KF].rearrange(
                            "p (g n c) -> p g n c", g=2, n=NO
                        ),
                        route16[:, g2 * 2 * CO : (g2 + 1) * 2 * CO]
                        .rearrange("p (g u c) -> p g u c", g=2, u=1)
                        .broadcast_to((128, 2, NO, CO)),
                    )
                    for q in range(8):
                        nc.tensor.matmul(
                            pre_ps2[:, (q % 4) * 512 : (q % 4 + 1) * 512],
                            ones_sb[:],
                            rv[:, q * 512 : (q + 1) * 512],
                            start=(g2 == 0 and q < 4),
                            stop=(g2 == G // 2 - 1 and q >= 4),
                            skip_group_check=True,
                        )
                pre_ps = pre_ps2
                allreduce_squash(it, 1.0)

        # final activation -> DRAM (partitions b*8, i.e. c8==0)
        nc.sync.dma_start(
            out_d, act32[:].rearrange("(b c) k -> b c k", c=8)[:, 0, :]
        )

    nc.compile()
    return nc


def _prep_inputs(x, weight, bias):
    """Per-core host-side shard prep. Returns list of in_maps."""
    w_nc = (
        weight.reshape(CI, NI, CO, NO)
        .transpose(0, 1, 3, 2)
        .reshape(CI * NI, NO * CO)
        .astype(np.float16)
    )
    bias_nc = np.ascontiguousarray(
        bias.reshape(CO, NO).T.reshape(NO * CO)
    ).astype(np.float32)
    in_maps = []
    for c in range(NCORES):
        xs = x[:, c * CIL : (c + 1) * CIL, :].astype(np.float16)  # [BS, CIL, NI]
        # xbd[p=(c8,ni), g, m=(b,c8')] block-diagonal
        xbd = np.zeros((128, G, 128), dtype=np.float16)
        xg = xs.reshape(BS, G, 8, NI)  # b g c8 ni
        for c8 in range(8):
            # rows c8*16..c8*16+16, cols b*8+c8
            xbd[c8 * NI : (c8 + 1) * NI, :, c8::8] = xg[:, :, c8, :].transpose(
                2, 1, 0
            )
        # xT[p=(c8,ni), g, b] = x[b, g*8+c8, ni]
        xT = np.ascontiguousarray(
            xg.transpose(2, 3, 1, 0).reshape(128, G, BS)
        ).astype(np.float16)
        ws = np.ascontiguousarray(
            w_nc[c * CIL * NI : (c + 1) * CIL * NI, :]
        )
        in_maps.append(
            {"xbd": xbd, "xT": xT, "w": ws, "bias": bias_nc}
        )
    return in_maps


def _run(inputs, trace=False, **kw):
    from concourse.bass_utils import run_bass_kernel_spmd

    if "nc" not in _CACHE:
        _CACHE["nc"] = _build_program()
    nc = _CACHE["nc"]
    in_maps = _prep_inputs(inputs["x"], inputs["weight"], inputs["bias"])
    res = run_bass_kernel_spmd(
        nc, in_maps, core_ids=list(range(NCORES)), trace=trace, **kw
    )
    out = res.results[0]["out"]  # [BS, NO*CO] in (n, co) order
    out = np.ascontiguousarray(
        out.reshape(BS, NO, CO).transpose(0, 2, 1)
    ).astype(np.float32)
    return out, res


def kernel(**inputs):
    out, _ = _run(inputs, trace=False)
    return out
